# revision 4
# baseline (speedup 1.0000x reference)
"""XNOR-Net++ style binary double-conv forward for Trainium2, 8 NeuronCores.

Computes, for x:[32,256,56,56] f32, w1,w2:[256,256,3,3] f32:
    alpha = (mean|w1| + mean|w2|)/2 over (I,kh,kw)            -> [O]
    out   = (conv(sign(x), sign(w1)) + conv(sign(x), sign(w2))) * alpha

Key algebraic fold: conv(s, sign(w1)) + conv(s, sign(w2)) ==
conv(s, sign(w1)+sign(w2)); the combined weight is in {-2,0,2} and the
activations in {-1,0,1}, both exact in fp8e4, so the conv's integer
part is reproduced exactly and only the final alpha multiply rounds.

Implementation: per output-row block, 9 shifted-tap matmuls accumulate
one PSUM tile (block-outer order). fp8 + DoubleRow folds the K=256
contraction (2x128 C_in chunks) into single matmuls. The padded sign
image is stored flat ([58*58] per C_in chunk, 58-wide rows, one guard
byte on each end) so every tap window is a contiguous 464-element
slice; the wrap-around columns land in the two padding columns of each
8-row output block and are dropped by the PSUM->SBUF scale-copy.

Scheduling (v2): the two HWDGE rings (sync, scalar) carry inputs in
exact consumption order -- oc0 weight chunks split in halves first,
then img0 in 28-row halves, then oc1 weights, then later images; the
SWDGE ring (gpsimd) carries all output stores. Weight sign runs on
the ACT engine per half-chunk as each DMA lands, the sign-sum on DVE,
alpha |w| reductions on gpsimd, and the 36 PE transposes are grouped
4-to-a-PSUM-bank so one batched cast drains each group. Convs run
block-outer so the PE stream starts as soon as the first image half
is signed, and per-block scale-copies alternate DVE/ACT with stores
riding the SWDGE ring.

Sharding: data-parallel over batch, 4 images per core, weights
replicated; outputs concatenated on host.
"""

import numpy as np

P = 128
H = W = 56
WP = 58  # padded row width
PLANE = WP * WP  # 3364 flat padded plane
PLANE_STRIDE = 3376  # 16-aligned (DoubleRow AP step%16==0), >= 1+PLANE+1
NIMG = 4  # images per core
NCORES = 8
RB = 8  # output rows per matmul block
NBLK = H // RB  # 7
N_FREE = RB * WP  # 464 <= 512 (one PSUM bank)
HR = 28  # x rows per DMA slice (half image)

_CACHE = {}


def _build_program():
    from contextlib import ExitStack

    import concourse.bacc as bacc
    import concourse.mybir as mybir
    import concourse.tile as tile
    from concourse.masks import make_identity

    dt = mybir.dt
    AF = mybir.ActivationFunctionType

    nc = bacc.Bacc(
        "TRN2",
        target_bir_lowering=False,
        debug=False,
        num_devices=NCORES,
    )
    x = nc.dram_tensor("x", [NIMG, 256, H, W], dt.float32, kind="ExternalInput").ap()
    w1 = nc.dram_tensor("w1", [256, 256, 3, 3], dt.float32, kind="ExternalInput").ap()
    w2 = nc.dram_tensor("w2", [256, 256, 3, 3], dt.float32, kind="ExternalInput").ap()
    out = nc.dram_tensor(
        "out", [NIMG, 256, H, W], dt.float32, kind="ExternalOutput"
    ).ap()

    with tile.TileContext(nc) as tc, ExitStack() as ctx:
        consts = ctx.enter_context(tc.tile_pool(name="consts", bufs=1))
        wraw = ctx.enter_context(tc.tile_pool(name="wraw", bufs=4))
        wsig = ctx.enter_context(tc.tile_pool(name="wsig", bufs=4))
        wsump = ctx.enter_context(tc.tile_pool(name="wsump", bufs=2))
        xraw_pool = ctx.enter_context(tc.tile_pool(name="xraw", bufs=8))
        xpad_pool = ctx.enter_context(tc.tile_pool(name="xpad", bufs=4))
        psum_t = ctx.enter_context(tc.tile_pool(name="psum_t", bufs=3, space="PSUM"))
        psum_c = ctx.enter_context(tc.tile_pool(name="psum_c", bufs=4, space="PSUM"))
        outp = ctx.enter_context(tc.tile_pool(name="outp", bufs=8))

        ident = consts.tile([P, P], dt.bfloat16)
        make_identity(nc, ident)

        # alpha per output channel, one column per oc chunk
        alpha_sb = consts.tile([P, 2], dt.float32)
        # stationary weights: per oc chunk, [i_local, tap, ic, o_local] fp8
        lhsT_all = [
            consts.tile([P, 9, 2, P], dt.float8e4, name=f"lhsT_{oc}") for oc in range(2)
        ]

        # ---- weight loads: halves so sign starts at half-DMA granularity ----
        wr = [[None, None], [None, None]]

        def load_w(oc):
            wr1 = wraw.tile([P, 256, 3, 3], dt.float32, tag="wraw", name=f"wr1_{oc}")
            wr2 = wraw.tile([P, 256, 3, 3], dt.float32, tag="wraw", name=f"wr2_{oc}")
            for h in range(2):
                ic0, ic1 = h * 128, (h + 1) * 128
                nc.sync.dma_start(
                    out=wr1[:, ic0:ic1], in_=w1[oc * P : (oc + 1) * P, ic0:ic1]
                )
                nc.scalar.dma_start(
                    out=wr2[:, ic0:ic1], in_=w2[oc * P : (oc + 1) * P, ic0:ic1]
                )
            wr[oc] = [wr1, wr2]

        def sign_w(oc):
            ws1 = wsig.tile([P, 256, 3, 3], dt.bfloat16, tag="wsig", name=f"ws1_{oc}")
            ws2 = wsig.tile([P, 256, 3, 3], dt.bfloat16, tag="wsig", name=f"ws2_{oc}")
            for ws, wrr in ((ws1, wr[oc][0]), (ws2, wr[oc][1])):
                for h in range(2):
                    ic0, ic1 = h * 128, (h + 1) * 128
                    nc.scalar.activation(
                        out=ws[:, ic0:ic1], in_=wrr[:, ic0:ic1], func=AF.Sign
                    )
            wsum = wsump.tile([P, 256, 3, 3], dt.bfloat16, tag="wsum", name=f"wsum_{oc}")
            nc.vector.tensor_add(out=wsum, in0=ws1, in1=ws2)
            return wsum

        def transpose_w(oc, wsum, use_vector_cast):
            # 9 taps x 2 ic = 18 [128,128] PE transposes, grouped 3 per PSUM
            # tile (within one ic) so one batched cast drains each group.
            for ic in range(2):
                for t0 in range(0, 9, 3):
                    pt = psum_t.tile(
                        [P, 3 * P], dt.bfloat16, tag="pt", name=f"pt_{oc}_{ic}_{t0}"
                    )
                    for k in range(3):
                        ky, kx = (t0 + k) // 3, (t0 + k) % 3
                        nc.tensor.transpose(
                            pt[:, k * P : (k + 1) * P],
                            wsum[:, ic * P : (ic + 1) * P, ky, kx],
                            ident,
                        )
                    dst = lhsT_all[oc][:, t0 : t0 + 3, ic, :]
                    src = pt[:, 0 : 3 * P].rearrange("p (g q) -> p g q", q=P)
                    if use_vector_cast:
                        nc.vector.tensor_copy(out=dst, in_=src)
                    else:
                        nc.scalar.activation(out=dst, in_=src, func=AF.Copy)

        def alpha_reduce(oc):
            # |w| partial sums per raw half-chunk (ready as each DMA lands),
            # then a tiny combine; all on DVE.
            parts = wraw.tile([P, 4], dt.float32, tag="asum", name=f"as_{oc}")
            k = 0
            for w_ in wr[oc]:
                for h in range(2):
                    nc.vector.tensor_reduce(
                        out=parts[:, k : k + 1],
                        in_=w_[:, h * 128 : (h + 1) * 128].rearrange(
                            "p a b c -> p (a b c)"
                        ),
                        axis=mybir.AxisListType.X,
                        op=mybir.AluOpType.add,
                        apply_absolute_value=True,
                    )
                    k += 1
            nc.vector.tensor_reduce(
                out=alpha_sb[:, oc : oc + 1],
                in_=parts,
                axis=mybir.AxisListType.X,
                op=mybir.AluOpType.add,
            )
            nc.vector.tensor_scalar_mul(
                alpha_sb[:, oc : oc + 1], alpha_sb[:, oc : oc + 1], 1.0 / (2 * 2304)
            )

        # ---- x streaming: padded-border memsets, then 28-row half DMAs ----
        xps = [None] * NIMG
        xrs = [[[None, None], [None, None]] for _ in range(NIMG)]  # [img][ic][half]

        def memset_xp(img):
            xp = xpad_pool.tile(
                [P, 2, PLANE_STRIDE], dt.float8e4, tag="xp", name=f"xp_{img}"
            )
            xps[img] = xp
            for ic in range(2):
                # zero only the padding borders: [guard+top row], [bottom row
                # +tail guard], and the adjacent (right,left) pad pairs
                # between consecutive interior rows.
                nc.gpsimd.memset(xp[:, ic, 0:59], 0.0)
                nc.gpsimd.memset(xp[:, ic, 3306:PLANE_STRIDE], 0.0)
                pairs = xp[:, ic, 58 : 58 + 56 * WP].rearrange(
                    "p (r w) -> p r w", w=WP
                )[:, :, 0:2]
                nc.gpsimd.memset(pairs, 0.0)

        def load_x(img):
            # consumption-ordered halves: (ic0,h) on sync, (ic1,h) on scalar
            for h in range(2):
                r0 = h * HR
                for ic, eng in ((0, nc.sync), (1, nc.scalar)):
                    xr = xraw_pool.tile(
                        [P, HR, W], dt.float32, tag="xr", name=f"xr_{img}_{ic}_{h}"
                    )
                    eng.dma_start(
                        out=xr, in_=x[img, ic * P : (ic + 1) * P, r0 : r0 + HR]
                    )
                    xrs[img][ic][h] = xr

        def sign_x(img, h):
            xp = xps[img]
            r0 = h * HR
            for ic in range(2):
                interior = xp[:, ic, 1 : 1 + PLANE].rearrange(
                    "p (h w) -> p h w", w=WP
                )[:, 1 + r0 : 1 + r0 + HR, 1 : 1 + W]
                nc.scalar.activation(
                    out=interior, in_=xrs[img][ic][h], func=AF.Sign
                )

        # ---- conv: block-outer, 9 accumulating taps per PSUM tile ----
        def conv_oc(img, oc):
            xp = xps[img]
            for blk in range(NBLK):
                pc = psum_c.tile(
                    [P, N_FREE], dt.float32, tag="acc", name=f"acc_{img}_{oc}_{blk}"
                )
                for tap in range(9):
                    ky, kx = tap // 3, tap % 3
                    win = (blk * RB + ky) * WP + kx
                    nc.tensor.matmul(
                        out=pc,
                        lhsT=lhsT_all[oc][:, tap],
                        rhs=xp[:, :, win : win + N_FREE],
                        start=(tap == 0),
                        stop=(tap == 8),
                        perf_mode=mybir.MatmulPerfMode.DoubleRow,
                    )
                rs = blk * RB
                ot = outp.tile([P, RB, W], dt.float32, tag="ot", name=f"ot_{img}_{oc}_{blk}")
                psv = pc[:].rearrange("p (h w) -> p h w", w=WP)[:, :, 1 : 1 + W]
                if blk % 2 == 0:
                    nc.vector.tensor_scalar_mul(ot, psv, alpha_sb[:, oc : oc + 1])
                else:
                    nc.scalar.activation(
                        out=ot, in_=psv, func=AF.Copy, scale=alpha_sb[:, oc : oc + 1]
                    )
                nc.gpsimd.dma_start(
                    out=out[img, oc * P : (oc + 1) * P, rs : rs + RB, :], in_=ot
                )

        # ---- schedule ----
        memset_xp(0)
        load_w(0)       # oc0 weight halves first on both HWDGE rings
        load_x(0)       # then img0 halves
        load_w(1)       # then oc1 weights
        wsum0 = sign_w(0)       # ACT signs fire per half as DMAs land
        alpha_reduce(0)         # gpsimd, off the critical path
        sign_x(0, 0)
        transpose_w(0, wsum0, True)
        sign_x(0, 1)
        memset_xp(1)
        load_x(1)
        conv_oc(0, 0)
        wsum1 = sign_w(1)
        alpha_reduce(1)
        transpose_w(1, wsum1, False)
        sign_x(1, 0)
        sign_x(1, 1)
        conv_oc(0, 1)
        memset_xp(2)
        load_x(2)
        sign_x(2, 0)
        sign_x(2, 1)
        conv_oc(1, 0)
        memset_xp(3)
        load_x(3)
        conv_oc(1, 1)
        sign_x(3, 0)
        sign_x(3, 1)
        conv_oc(2, 0)
        conv_oc(2, 1)
        conv_oc(3, 0)
        conv_oc(3, 1)

    nc.compile()
    return nc


def _get_program():
    if "nc" not in _CACHE:
        _CACHE["nc"] = _build_program()
    return _CACHE["nc"]


def _run(x, weight1, weight2, **spmd_kwargs):
    from concourse.bass_utils import run_bass_kernel_spmd

    nc = _get_program()
    x = np.ascontiguousarray(x, dtype=np.float32)
    w1 = np.ascontiguousarray(weight1, dtype=np.float32)
    w2 = np.ascontiguousarray(weight2, dtype=np.float32)
    in_maps = [
        {"x": x[i * NIMG : (i + 1) * NIMG], "w1": w1, "w2": w2} for i in range(NCORES)
    ]
    res = run_bass_kernel_spmd(nc, in_maps, list(range(NCORES)), **spmd_kwargs)
    out = np.concatenate([res.results[i]["out"] for i in range(NCORES)], axis=0)
    return out, res


def kernel(x, weight1, weight2):
    out, _ = _run(x, weight1, weight2)
    return out


# revision 8
# speedup vs baseline: 1.0145x; 1.0145x over previous
"""XNOR-Net++ style binary double-conv forward for Trainium2, 8 NeuronCores.

Computes, for x:[32,256,56,56] f32, w1,w2:[256,256,3,3] f32:
    alpha = (mean|w1| + mean|w2|)/2 over (I,kh,kw)            -> [O]
    out   = (conv(sign(x), sign(w1)) + conv(sign(x), sign(w2))) * alpha

Key algebraic fold: conv(s, sign(w1)) + conv(s, sign(w2)) ==
conv(s, sign(w1)+sign(w2)); the combined weight is in {-2,0,2} and the
activations in {-1,0,1}, both exact in fp8e4, so the conv's integer
part is reproduced exactly and only the final alpha multiply rounds.

Implementation: per output-row block, 9 shifted-tap matmuls accumulate
one PSUM tile (block-outer order). fp8 + DoubleRow folds the K=256
contraction (2x128 C_in chunks) into single matmuls. The padded sign
image is stored flat ([58*58] per C_in chunk, 58-wide rows, one guard
byte on each end) so every tap window is a contiguous 464-element
slice; the wrap-around columns land in the two padding columns of each
8-row output block and are dropped by the PSUM->SBUF scale-copy.

Scheduling (v2): the two HWDGE rings (sync, scalar) carry inputs in
exact consumption order -- oc0 weight chunks split in halves first,
then img0 in 28-row halves, then oc1 weights, then later images; the
SWDGE ring (gpsimd) carries all output stores. Weight sign runs on
the ACT engine per half-chunk as each DMA lands, the sign-sum on DVE,
alpha |w| reductions on gpsimd, and the 36 PE transposes are grouped
4-to-a-PSUM-bank so one batched cast drains each group. Convs run
block-outer so the PE stream starts as soon as the first image half
is signed, and per-block scale-copies alternate DVE/ACT with stores
riding the SWDGE ring.

Sharding: data-parallel over batch, 4 images per core, weights
replicated; outputs concatenated on host.
"""

import numpy as np

P = 128
H = W = 56
WP = 58  # padded row width
PLANE = WP * WP  # 3364 flat padded plane
PLANE_STRIDE = 3376  # 16-aligned (DoubleRow AP step%16==0), >= 1+PLANE+1
NIMG = 4  # images per core
NCORES = 8
RB = 8  # output rows per matmul block
NBLK = H // RB  # 7
N_FREE = RB * WP  # 464 <= 512 (one PSUM bank)
HR = 28  # x rows per DMA slice (half image)

_CACHE = {}


def _build_program():
    from contextlib import ExitStack

    import concourse.bacc as bacc
    import concourse.mybir as mybir
    import concourse.tile as tile
    from concourse.masks import make_identity

    dt = mybir.dt
    AF = mybir.ActivationFunctionType

    nc = bacc.Bacc(
        "TRN2",
        target_bir_lowering=False,
        debug=False,
        num_devices=NCORES,
    )
    x = nc.dram_tensor("x", [NIMG, 256, H, W], dt.float32, kind="ExternalInput").ap()
    w1 = nc.dram_tensor("w1", [256, 256, 3, 3], dt.float32, kind="ExternalInput").ap()
    w2 = nc.dram_tensor("w2", [256, 256, 3, 3], dt.float32, kind="ExternalInput").ap()
    out = nc.dram_tensor(
        "out", [NIMG, 256, H, W], dt.float32, kind="ExternalOutput"
    ).ap()

    with tile.TileContext(nc) as tc, ExitStack() as ctx:
        consts = ctx.enter_context(tc.tile_pool(name="consts", bufs=1))
        wraw = ctx.enter_context(tc.tile_pool(name="wraw", bufs=4))
        wsig = ctx.enter_context(tc.tile_pool(name="wsig", bufs=4))
        wsump = ctx.enter_context(tc.tile_pool(name="wsump", bufs=2))
        xraw_pool = ctx.enter_context(tc.tile_pool(name="xraw", bufs=6))
        xpad_pool = ctx.enter_context(tc.tile_pool(name="xpad", bufs=4))
        psum_t = ctx.enter_context(tc.tile_pool(name="psum_t", bufs=3, space="PSUM"))
        psum_c = ctx.enter_context(tc.tile_pool(name="psum_c", bufs=4, space="PSUM"))
        outp = ctx.enter_context(tc.tile_pool(name="outp", bufs=3))

        ident = consts.tile([P, P], dt.bfloat16)
        make_identity(nc, ident)

        # alpha per output channel, one column per oc chunk
        alpha_sb = consts.tile([P, 2], dt.float32)
        # stationary weights: per oc chunk, [i_local, tap, ic, o_local] fp8
        lhsT_all = [
            consts.tile([P, 9, 2, P], dt.float8e4, name=f"lhsT_{oc}") for oc in range(2)
        ]

        # ---- weight loads: halves so sign starts at half-DMA granularity ----
        wr = [[None, None], [None, None]]

        def load_w(oc):
            wr1 = wraw.tile([P, 256, 3, 3], dt.float32, tag="wraw", name=f"wr1_{oc}")
            wr2 = wraw.tile([P, 256, 3, 3], dt.float32, tag="wraw", name=f"wr2_{oc}")
            for h in range(2):
                ic0, ic1 = h * 128, (h + 1) * 128
                nc.sync.dma_start(
                    out=wr1[:, ic0:ic1], in_=w1[oc * P : (oc + 1) * P, ic0:ic1]
                )
                nc.scalar.dma_start(
                    out=wr2[:, ic0:ic1], in_=w2[oc * P : (oc + 1) * P, ic0:ic1]
                )
            wr[oc] = [wr1, wr2]

        def sign_w(oc):
            ws1 = wsig.tile([P, 256, 3, 3], dt.bfloat16, tag="wsig", name=f"ws1_{oc}")
            ws2 = wsig.tile([P, 256, 3, 3], dt.bfloat16, tag="wsig", name=f"ws2_{oc}")
            for ws, wrr in ((ws1, wr[oc][0]), (ws2, wr[oc][1])):
                for h in range(2):
                    ic0, ic1 = h * 128, (h + 1) * 128
                    nc.scalar.activation(
                        out=ws[:, ic0:ic1], in_=wrr[:, ic0:ic1], func=AF.Sign
                    )
            wsum = wsump.tile([P, 256, 3, 3], dt.bfloat16, tag="wsum", name=f"wsum_{oc}")
            nc.vector.tensor_add(out=wsum, in0=ws1, in1=ws2)
            return wsum

        def transpose_w(oc, wsum, alternate_cast):
            # 9 taps x 2 ic = 18 [128,128] PE transposes, grouped 3 per PSUM
            # tile (within one ic) so one batched cast drains each group.
            g = 0
            for ic in range(2):
                for t0 in range(0, 9, 3):
                    pt = psum_t.tile(
                        [P, 3 * P], dt.bfloat16, tag="pt", name=f"pt_{oc}_{ic}_{t0}"
                    )
                    for k in range(3):
                        ky, kx = (t0 + k) // 3, (t0 + k) % 3
                        nc.tensor.transpose(
                            pt[:, k * P : (k + 1) * P],
                            wsum[:, ic * P : (ic + 1) * P, ky, kx],
                            ident,
                        )
                    dst = lhsT_all[oc][:, t0 : t0 + 3, ic, :]
                    src = pt[:, 0 : 3 * P].rearrange("p (g q) -> p g q", q=P)
                    if alternate_cast and g % 2 == 1:
                        nc.scalar.activation(out=dst, in_=src, func=AF.Copy)
                    else:
                        nc.vector.tensor_copy(out=dst, in_=src)
                    g += 1

        def alpha_reduce(oc):
            # |w| partial sums per raw half-chunk (ready as each DMA lands),
            # then a tiny combine; all on DVE.
            parts = wraw.tile([P, 4], dt.float32, tag="asum", name=f"as_{oc}")
            k = 0
            for w_ in wr[oc]:
                for h in range(2):
                    nc.vector.tensor_reduce(
                        out=parts[:, k : k + 1],
                        in_=w_[:, h * 128 : (h + 1) * 128].rearrange(
                            "p a b c -> p (a b c)"
                        ),
                        axis=mybir.AxisListType.X,
                        op=mybir.AluOpType.add,
                        apply_absolute_value=True,
                    )
                    k += 1
            nc.vector.tensor_reduce(
                out=alpha_sb[:, oc : oc + 1],
                in_=parts,
                axis=mybir.AxisListType.X,
                op=mybir.AluOpType.add,
            )
            nc.vector.tensor_scalar_mul(
                alpha_sb[:, oc : oc + 1], alpha_sb[:, oc : oc + 1], 1.0 / (2 * 2304)
            )

        # ---- x streaming: padded-border memsets, then 28-row half DMAs ----
        xps = [None] * NIMG
        xrs = [[[None, None], [None, None]] for _ in range(NIMG)]  # [img][ic][half]

        def memset_xp(img):
            xp = xpad_pool.tile(
                [P, 2, PLANE_STRIDE], dt.float8e4, tag="xp", name=f"xp_{img}"
            )
            xps[img] = xp
            for ic in range(2):
                # zero only the padding borders: [guard+top row], [bottom row
                # +tail guard], and the adjacent (right,left) pad pairs
                # between consecutive interior rows.
                nc.gpsimd.memset(xp[:, ic, 0:59], 0.0)
                nc.gpsimd.memset(xp[:, ic, 3306:PLANE_STRIDE], 0.0)
                pairs = xp[:, ic, 58 : 58 + 56 * WP].rearrange(
                    "p (r w) -> p r w", w=WP
                )[:, :, 0:2]
                nc.gpsimd.memset(pairs, 0.0)

        def load_x(img):
            # consumption-ordered halves, all on the SWDGE ring so the two
            # HWDGE rings are free for weights at startup / outputs later
            for h in range(2):
                r0 = h * HR
                for ic in range(2):
                    xr = xraw_pool.tile(
                        [P, HR, W], dt.float32, tag="xr", name=f"xr_{img}_{ic}_{h}"
                    )
                    nc.gpsimd.dma_start(
                        out=xr, in_=x[img, ic * P : (ic + 1) * P, r0 : r0 + HR]
                    )
                    xrs[img][ic][h] = xr

        def sign_x(img, h):
            xp = xps[img]
            r0 = h * HR
            for ic in range(2):
                interior = xp[:, ic, 1 : 1 + PLANE].rearrange(
                    "p (h w) -> p h w", w=WP
                )[:, 1 + r0 : 1 + r0 + HR, 1 : 1 + W]
                nc.scalar.activation(
                    out=interior, in_=xrs[img][ic][h], func=AF.Sign
                )

        # ---- conv: block-outer, 9 accumulating taps per PSUM tile ----
        out_ring = [0]
        rings = (nc.sync, nc.scalar, nc.gpsimd)

        def conv_oc(img, oc):
            xp = xps[img]
            # one whole-plane output tile per (img, oc): per-block copies
            # land in row slices, then two big-descriptor DMAs store it
            ot = outp.tile([P, H, W], dt.float32, tag="ot", name=f"ot_{img}_{oc}")
            for blk in range(NBLK):
                pc = psum_c.tile(
                    [P, N_FREE], dt.float32, tag="acc", name=f"acc_{img}_{oc}_{blk}"
                )
                for tap in range(9):
                    ky, kx = tap // 3, tap % 3
                    win = (blk * RB + ky) * WP + kx
                    nc.tensor.matmul(
                        out=pc,
                        lhsT=lhsT_all[oc][:, tap],
                        rhs=xp[:, :, win : win + N_FREE],
                        start=(tap == 0),
                        stop=(tap == 8),
                        perf_mode=mybir.MatmulPerfMode.DoubleRow,
                    )
                rs = blk * RB
                psv = pc[:].rearrange("p (h w) -> p h w", w=WP)[:, :, 1 : 1 + W]
                dst = ot[:, rs : rs + RB, :]
                if blk % 2 == 0:
                    nc.vector.tensor_scalar_mul(dst, psv, alpha_sb[:, oc : oc + 1])
                else:
                    nc.scalar.activation(
                        out=dst, in_=psv, func=AF.Copy, scale=alpha_sb[:, oc : oc + 1]
                    )
                if blk == 3 or blk == NBLK - 1:
                    r0, r1 = (0, 32) if blk == 3 else (32, H)
                    eng = rings[out_ring[0] % 3]
                    out_ring[0] += 1
                    eng.dma_start(
                        out=out[img, oc * P : (oc + 1) * P, r0:r1, :],
                        in_=ot[:, r0:r1, :],
                    )

        # ---- schedule ----
        memset_xp(0)
        load_w(0)       # oc0 weight halves first on both HWDGE rings
        load_x(0)       # then img0 halves
        load_w(1)       # then oc1 weights
        wsum0 = sign_w(0)       # ACT signs fire per half as DMAs land
        alpha_reduce(0)         # gpsimd, off the critical path
        sign_x(0, 0)
        transpose_w(0, wsum0, False)
        sign_x(0, 1)
        memset_xp(1)
        load_x(1)
        conv_oc(0, 0)
        wsum1 = sign_w(1)
        alpha_reduce(1)
        transpose_w(1, wsum1, True)
        sign_x(1, 0)
        sign_x(1, 1)
        conv_oc(0, 1)
        memset_xp(2)
        load_x(2)
        sign_x(2, 0)
        sign_x(2, 1)
        conv_oc(1, 0)
        memset_xp(3)
        load_x(3)
        conv_oc(1, 1)
        sign_x(3, 0)
        sign_x(3, 1)
        conv_oc(2, 0)
        conv_oc(2, 1)
        conv_oc(3, 0)
        conv_oc(3, 1)

    nc.compile()
    return nc


def _get_program():
    if "nc" not in _CACHE:
        _CACHE["nc"] = _build_program()
    return _CACHE["nc"]


def _run(x, weight1, weight2, **spmd_kwargs):
    from concourse.bass_utils import run_bass_kernel_spmd

    nc = _get_program()
    x = np.ascontiguousarray(x, dtype=np.float32)
    w1 = np.ascontiguousarray(weight1, dtype=np.float32)
    w2 = np.ascontiguousarray(weight2, dtype=np.float32)
    in_maps = [
        {"x": x[i * NIMG : (i + 1) * NIMG], "w1": w1, "w2": w2} for i in range(NCORES)
    ]
    res = run_bass_kernel_spmd(nc, in_maps, list(range(NCORES)), **spmd_kwargs)
    out = np.concatenate([res.results[i]["out"] for i in range(NCORES)], axis=0)
    return out, res


def kernel(x, weight1, weight2):
    out, _ = _run(x, weight1, weight2)
    return out
